# revision 15
# baseline (speedup 1.0000x reference)
"""Cross-attention block (q from z_hsi, k/v from z_msi, softmax over 6400
pixels, residual + gamma) on 8 Trainium2 NeuronCores.

Sharding: the (batch=2, N=6400) query-pixel space is split into 8 shards of
1600 pixels (4 shards per batch element). Each core computes its shard's
attention output against the full key/value set of its batch element; the
host slices inputs and concatenates outputs (no device collectives).

Math restructuring vs the naive form:
  * softmax over j is invariant to adding any per-i constant, so the K bias
    (bk) is dropped entirely, and
      E[j,i] = K[:,j]^T Q[:,i]  ==  zm[:,j]^T (Wk^T Wq zq + Wk^T bq)[:,i]
    so K and Q are never materialized: one [64 x 1600] "QK" projection
    (host precomputes Wq^T Wk and Wk^T bq) replaces both.
  * The V bias folds out of the attention matmul:  (V+bv) P = V P + bv * d,
    so after normalizing by d it becomes "+ gamma*bv" on the residual;
    gamma itself is folded into Wv on the host, so the device only ever
    divides by the raw denominator d.
  * E/QK matmuls run in float32r (TF32-like precision at full PE rate,
    1 cycle/row for moving widths >= 256).  exp() output is bfloat16:
    ACT throughput is dtype-independent, but bf16 gives the PV matmul a
    1 cycle/row moving operand at ANY width and doubles DVE throughput
    (2x_1p mode) for the denominator accumulation.
  * All K=64 contractions (z_msi channels) are zero-padded to K=128.
  * Denominator: instead of one ones-matmul per exp pair (~17us of PE),
    two bf16 DVE chain-accumulators per i-block sum the 50 exp tiles
    elementwise; a single pair of matmuls against an all-ones [128,128]
    stationary then reduces over partitions AND broadcasts d[i] to all
    128 output partitions in one shot (so no GPSIMD broadcast either).
  * exp runs on ACT straight out of PSUM in [128, 2x400] strided batches;
    PV matmuls trail three exp-groups behind (software pipeline).  VT
    tiles (V^T = zm^T Wv) are computed on the PE interleaved into block
    0's group loop; their PSUM->SBUF(bf16) copies run on the otherwise
    idle GPSIMD/Pool engine.
"""
import sys

sys.path.insert(0, "/opt/trn_rl_repo")

import ml_dtypes
import numpy as np
import concourse.bass as bass  # noqa: F401
import concourse.tile as tile
from concourse import bacc, mybir
from concourse.bass_utils import run_bass_kernel_spmd

B, CH, CM, CO = 2, 128, 64, 128
H = W = 80
N = H * W                # 6400 key/value pixels per batch element
NCORES = 8
NI = (B * N) // NCORES   # 1600 query pixels per core
JT = N // 128            # 50 key tiles
F32 = mybir.dt.float32
F32R = mybir.dt.float32r
BF16 = mybir.dt.bfloat16

IBS = 400
I_BLOCKS = [(k * IBS, IBS) for k in range(NI // IBS)]
# jt groups of 3: one exp instruction per group
GROUPS = [tuple(range(g, min(g + 3, JT))) for g in range(0, JT, 3)]
NCHAIN = 3


def _build(repeat=1, sim_unroll=1):
    """repeat>1 wraps the per-core compute in an on-device For_i loop (two
    body copies per iteration); used only by the perf harness to measure HW
    time via wall-clock slope.  sim_unroll emits extra python-unrolled body
    copies at repeat=1 for TimelineSim marginal-cost analysis."""
    nc = bacc.Bacc(None, target_bir_lowering=False)
    zq = nc.declare_dram_parameter("zq", [CH, NI], F32R, isOutput=False)
    zm = nc.declare_dram_parameter("zm", [128, N], F32R, isOutput=False)
    wqk = nc.declare_dram_parameter("wqk", [CH, 128], F32R, isOutput=False)
    bkq = nc.declare_dram_parameter("bkq", [128, 1], F32, isOutput=False)
    wvT = nc.declare_dram_parameter("wvT", [128, CO], BF16, isOutput=False)
    zmb = nc.declare_dram_parameter("zmb", [128, N], BF16, isOutput=False)
    gbv = nc.declare_dram_parameter("gbv", [CO, 1], F32, isOutput=False)
    ones = nc.declare_dram_parameter("ones", [128, 128], BF16, isOutput=False)
    out = nc.declare_dram_parameter("out", [CO, NI], F32, isOutput=True)

    with tile.TileContext(nc) as tc:
        with (
            tc.tile_pool(name="big", bufs=1) as big,
            tc.tile_pool(name="dbl", bufs=2) as dbl,
            tc.tile_pool(name="expp", bufs=6) as expp,
            tc.tile_pool(name="work", bufs=2) as work,
            tc.tile_pool(name="pse", bufs=2, space="PSUM") as pse,
            tc.tile_pool(name="pspv", bufs=1, space="PSUM") as pspv,
        ):
            zm_sb = big.tile([128, N], F32R)
            nc.sync.dma_start(zm_sb[:], zm[:])
            zq_sb = big.tile([CH, NI], F32R)
            nc.sync.dma_start(zq_sb[:], zq[:])
            wqk_sb = big.tile([CH, 128], F32R)
            nc.sync.dma_start(wqk_sb[:], wqk[:])
            bkq_sb = big.tile([128, 1], F32)
            nc.sync.dma_start(bkq_sb[:], bkq[:])
            wv_sb = big.tile([128, CO], BF16)
            nc.sync.dma_start(wv_sb[:], wvT[:])
            zmb_sb = big.tile([128, N], BF16)
            nc.sync.dma_start(zmb_sb[:], zmb[:])
            gbv_sb = big.tile([CO, 1], F32)
            nc.sync.dma_start(gbv_sb[:], gbv[:])
            ones_sb = big.tile([128, 128], BF16)
            nc.sync.dma_start(ones_sb[:], ones[:])

            # Two body copies per For_i iteration: the "dbl" pool rotates
            # between the copies, so consecutive iterations overlap (the
            # hardware loop replays a static instruction stream -- a single
            # body would serialize on its own head/tail tiles).
            from contextlib import nullcontext
            if repeat > 1:
                assert repeat % 2 == 0
                rep_ctx = tc.For_i(0, repeat // 2, 1)
                n_emit = 2
            else:
                rep_ctx = nullcontext()
                n_emit = sim_unroll
            with rep_ctx:
                for _ in range(n_emit):
                    _emit_body(nc, tc, dbl, expp, work, pse, pspv,
                               zm_sb, zmb_sb, zq_sb, wqk_sb, bkq_sb,
                               wv_sb, gbv_sb, ones_sb, out)

    nc.finalize()
    return nc


def _emit_body(nc, tc, dbl, expp, work, pse, pspv,
               zm_sb, zmb_sb, zq_sb, wqk_sb, bkq_sb, wv_sb,
               gbv_sb, ones_sb, out):
    # residual (+ folded gamma*bv), exact fp32 bits of z_hsi
    zqp = dbl.tile([CH, NI], F32, tag="zqp")
    nc.vector.tensor_scalar_add(zqp[:], zq_sb[:].bitcast(F32), gbv_sb[:])

    # QK[c, i] = (Wk^T Wq zq + Wk^T bq)[c, i]   -> E = zm^T QK
    QK_sb = dbl.tile([128, NI], F32R, tag="qk")
    for c0 in range(0, NI, 400):
        cs = min(400, NI - c0)
        pq = pse.tile([128, 1536], F32, tag="e")
        nc.tensor.matmul(pq[:, :cs], wqk_sb[:], zq_sb[:, c0:c0 + cs],
                         start=True, stop=True)
        nc.vector.tensor_scalar_add(QK_sb[:, c0:c0 + cs], pq[:, :cs],
                                    bkq_sb[:])

    # VT tiles: VT[j, o] = sum_c zm[c, j] (gamma*Wv)[o, c] -- computed
    # interleaved into block 0's group loop (quad q lands at group q, always
    # ahead of the lag-3 PV consumer of tiles 3g..3g+2), so ACT's exp chain
    # starts immediately instead of idling behind a PE-only prefix.
    VT_sb = dbl.tile([128, JT * CO], BF16, tag="vt")

    # main attention loop; PV matmuls trail three exp-groups behind
    for i0, ibs in I_BLOCKS:
        pv = pspv.tile([128, 512], F32, tag="pv")
        sacc = [work.tile([128, 512], BF16, tag=f"s{c}", name=f"sacc{c}")
                for c in range(NCHAIN)]

        def emit_pv(p3_prev, gi):
            for t, jt in enumerate(GROUPS[gi]):
                nc.tensor.matmul(
                    pv[:, :ibs],
                    VT_sb[:, jt * 128:(jt + 1) * 128],
                    p3_prev[:, t * 512:t * 512 + ibs],
                    start=(jt == 0), stop=(jt == JT - 1),
                    skip_group_check=True)

        from collections import deque
        pending = deque()
        for gi, grp in enumerate(GROUPS):
            m = len(grp)
            e3 = pse.tile([128, 1536], F32, tag="e")
            for t, jt in enumerate(grp):
                nc.tensor.matmul(
                    e3[:, t * 512:t * 512 + ibs],
                    zm_sb[:, jt * 128:(jt + 1) * 128],
                    QK_sb[:, i0:i0 + ibs],
                    start=True, stop=True)
            p3 = expp.tile([128, 1536], BF16, tag="p")
            e3v = e3[:].rearrange("p (t x) -> p t x", x=512)[:, :m, :ibs]
            p3v = p3[:].rearrange("p (t x) -> p t x", x=512)[:, :m, :ibs]
            nc.scalar.activation(p3v, e3v, mybir.ActivationFunctionType.Exp)
            # denominator chains: NCHAIN independent bf16 accumulators
            for t in range(m):
                seg = p3[:, t * 512:t * 512 + ibs]
                if gi == 0:
                    nc.vector.tensor_copy(sacc[t][:, :ibs], seg)
                else:
                    nc.vector.tensor_add(sacc[t][:, :ibs],
                                         sacc[t][:, :ibs], seg)
            if i0 == 0 and gi * 4 < JT:
                nq = min(4, JT - gi * 4)
                vtq = pspv.tile([128, 512], F32, tag="d")
                for jj in range(nq):
                    j0 = (gi * 4 + jj) * 128
                    nc.tensor.matmul(vtq[:, jj * 128:(jj + 1) * 128],
                                     zmb_sb[:, j0:j0 + 128], wv_sb[:],
                                     start=True, stop=True,
                                     skip_group_check=True)
                nc.vector.tensor_copy(
                    VT_sb[:, gi * 512:gi * 512 + nq * 128],
                    vtq[:, :nq * 128])
            pending.append((p3, gi))
            if len(pending) > 3:
                emit_pv(*pending.popleft())
        while pending:
            emit_pv(*pending.popleft())

        # d[i] = sum_j exp: matmuls against the all-ones stationary reduce
        # the chains over partitions AND broadcast d[i] to all 128 output
        # partitions in one shot.
        dbc = pspv.tile([128, 512], F32, tag="d")
        for c in range(NCHAIN):
            nc.tensor.matmul(dbc[:, :ibs], ones_sb[:], sacc[c][:, :ibs],
                             start=(c == 0), stop=(c == NCHAIN - 1),
                             skip_group_check=True)

        # normalize: out = PV / d + zqp   (gamma pre-folded into Wv)
        r_sb = work.tile([128, 512], F32, tag="r")
        nc.vector.reciprocal(r_sb[:, :ibs], dbc[:, :ibs])
        t_sb = work.tile([128, 512], F32, tag="t")
        nc.vector.tensor_mul(t_sb[:, :ibs], pv[:, :ibs], r_sb[:, :ibs])
        o_sb = work.tile([128, 512], F32, tag="o")
        nc.vector.tensor_add(o_sb[:, :ibs], t_sb[:, :ibs],
                             zqp[:, i0:i0 + ibs])
        nc.sync.dma_start(out[:, i0:i0 + ibs], o_sb[:, :ibs])


_cached_nc = None


def kernel(z_hsi, z_msi, Wq, bq, Wk, bk, Wv, bv, gamma):
    global _cached_nc
    if _cached_nc is None:
        _cached_nc = _build()
    nc = _cached_nc

    z_hsi = np.asarray(z_hsi, dtype=np.float32).reshape(B, CH, N)
    z_msi = np.ascontiguousarray(np.asarray(z_msi, dtype=np.float32).reshape(B, CM, N))
    Wq64 = np.asarray(Wq, dtype=np.float64)
    Wk64 = np.asarray(Wk, dtype=np.float64)
    bq64 = np.asarray(bq, dtype=np.float64)
    # QK folding: E = zm^T (Wk^T Wq zq + Wk^T bq); bk cancels in softmax.
    # All CM=64 contractions are zero-padded to 128: K=64 matmuls run ~2x
    # slower per column on TRN2 than K=128.
    wqk_h = np.zeros((CH, 128), np.float32)
    wqk_h[:, :CM] = (Wq64.T @ Wk64).astype(np.float32)
    bkq_h = np.zeros((128, 1), np.float32)
    bkq_h[:CM, 0] = (Wk64.T @ bq64).astype(np.float32)
    g = float(np.asarray(gamma, dtype=np.float32).reshape(-1)[0])
    # gamma folds into Wv (bf16): out = (g*Wv) zm P / d + (z_hsi + g*bv)
    wvT_h = np.zeros((128, CO), ml_dtypes.bfloat16)
    wvT_h[:CM] = (g * np.asarray(Wv, np.float64).T).astype(np.float32)
    z_msi_pad = np.zeros((B, 128, N), np.float32)
    z_msi_pad[:, :CM] = z_msi
    z_msi_bf = z_msi_pad.astype(ml_dtypes.bfloat16)
    gbv = np.ascontiguousarray((g * np.asarray(bv, np.float32)).reshape(CO, 1))
    ones = np.ones((128, 128), dtype=ml_dtypes.bfloat16)

    shards_per_b = NCORES // B
    in_maps = []
    for c in range(NCORES):
        b, s = c // shards_per_b, (c % shards_per_b) * NI
        in_maps.append({
            "zq": np.ascontiguousarray(z_hsi[b][:, s:s + NI]),
            "zm": z_msi_pad[b], "zmb": z_msi_bf[b],
            "wqk": wqk_h, "bkq": bkq_h, "wvT": wvT_h,
            "gbv": gbv, "ones": ones,
        })

    res = run_bass_kernel_spmd(nc, in_maps, core_ids=list(range(NCORES)))

    out = np.empty((B, CH, N), dtype=np.float32)
    for c in range(NCORES):
        b, s = c // shards_per_b, (c % shards_per_b) * NI
        out[b][:, s:s + NI] = res.results[c]["out"]
    return out.reshape(B, CH, H, W)


# revision 16
# speedup vs baseline: 1.4273x; 1.4273x over previous
"""Cross-attention block (q from z_hsi, k/v from z_msi, softmax over 6400
pixels, residual + gamma) on 8 Trainium2 NeuronCores.

Sharding: the (batch=2, N=6400) query-pixel space is split into 8 shards of
1600 pixels (4 shards per batch element). Each core computes its shard's
attention output against the full key/value set of its batch element; the
host slices inputs and concatenates outputs (no device collectives).

Math restructuring vs the naive form:
  * softmax over j is invariant to adding any per-i constant, so the K bias
    (bk) is dropped entirely, and
      E[j,i] = K[:,j]^T Q[:,i]  ==  zm[:,j]^T (Wk^T Wq zq + Wk^T bq)[:,i]
    so K and Q are never materialized: one [64 x 1600] "QK" projection
    (host precomputes Wq^T Wk and Wk^T bq) replaces both.
  * The V bias folds out of the attention matmul:  (V+bv) P = V P + bv * d,
    so after normalizing by d it becomes "+ gamma*bv" on the residual;
    gamma itself is folded into Wv on the host, so the device only ever
    divides by the raw denominator d.
  * E/QK matmuls run in float32r (TF32-like precision at full PE rate,
    1 cycle/row for moving widths >= 256).  exp() output is bfloat16:
    ACT throughput is dtype-independent, but bf16 gives the PV matmul a
    1 cycle/row moving operand at ANY width and doubles DVE throughput
    (2x_1p mode) for the denominator accumulation.
  * All K=64 contractions (z_msi channels) are zero-padded to K=128.
  * Denominator: instead of one ones-matmul per exp pair (~17us of PE),
    two bf16 DVE chain-accumulators per i-block sum the 50 exp tiles
    elementwise; a single pair of matmuls against an all-ones [128,128]
    stationary then reduces over partitions AND broadcasts d[i] to all
    128 output partitions in one shot (so no GPSIMD broadcast either).
  * exp runs on ACT straight out of PSUM in [128, 2x400] strided batches;
    PV matmuls trail three exp-groups behind (software pipeline).  VT
    tiles (V^T = zm^T Wv) are computed on the PE interleaved into block
    0's group loop; their PSUM->SBUF(bf16) copies run on the otherwise
    idle GPSIMD/Pool engine.
"""
import sys

sys.path.insert(0, "/opt/trn_rl_repo")

import ml_dtypes
import numpy as np
import concourse.bass as bass  # noqa: F401
import concourse.tile as tile
from concourse import bacc, mybir
from concourse.bass_utils import run_bass_kernel_spmd

B, CH, CM, CO = 2, 128, 64, 128
H = W = 80
N = H * W                # 6400 key/value pixels per batch element
NCORES = 8
NI = (B * N) // NCORES   # 1600 query pixels per core
JT = N // 128            # 50 key tiles
F32 = mybir.dt.float32
F32R = mybir.dt.float32r
BF16 = mybir.dt.bfloat16

IBS = 400
I_BLOCKS = [(k * IBS, IBS) for k in range(NI // IBS)]
# jt groups of 3: one exp instruction per group
GROUPS = [tuple(range(g, min(g + 3, JT))) for g in range(0, JT, 3)]
NCHAIN = 3


def _build(repeat=1, sim_unroll=1):
    """repeat>1 wraps the per-core compute in an on-device For_i loop (two
    body copies per iteration); used only by the perf harness to measure HW
    time via wall-clock slope.  sim_unroll emits extra python-unrolled body
    copies at repeat=1 for TimelineSim marginal-cost analysis."""
    nc = bacc.Bacc(None, target_bir_lowering=False)
    zq = nc.declare_dram_parameter("zq", [CH, NI], F32R, isOutput=False)
    zm = nc.declare_dram_parameter("zm", [128, N], F32R, isOutput=False)
    wqk = nc.declare_dram_parameter("wqk", [CH, 128], F32R, isOutput=False)
    bkq = nc.declare_dram_parameter("bkq", [128, 1], F32, isOutput=False)
    wvT = nc.declare_dram_parameter("wvT", [128, CO], BF16, isOutput=False)
    zmb = nc.declare_dram_parameter("zmb", [128, N], BF16, isOutput=False)
    gbv = nc.declare_dram_parameter("gbv", [CO, 1], F32, isOutput=False)
    ones = nc.declare_dram_parameter("ones", [128, 128], BF16, isOutput=False)
    out = nc.declare_dram_parameter("out", [CO, NI], F32, isOutput=True)

    with tile.TileContext(nc) as tc:
        with (
            tc.tile_pool(name="big", bufs=1) as big,
            tc.tile_pool(name="dbl", bufs=2) as dbl,
            tc.tile_pool(name="expp", bufs=6) as expp,
            tc.tile_pool(name="work", bufs=2) as work,
            tc.tile_pool(name="pse", bufs=2, space="PSUM") as pse,
            tc.tile_pool(name="pspv", bufs=1, space="PSUM") as pspv,
        ):
            zm_sb = big.tile([128, N], F32R)
            nc.sync.dma_start(zm_sb[:], zm[:])
            zq_sb = big.tile([CH, NI], F32R)
            nc.sync.dma_start(zq_sb[:], zq[:])
            wqk_sb = big.tile([CH, 128], F32R)
            nc.sync.dma_start(wqk_sb[:], wqk[:])
            bkq_sb = big.tile([128, 1], F32)
            nc.sync.dma_start(bkq_sb[:], bkq[:])
            wv_sb = big.tile([128, CO], BF16)
            nc.sync.dma_start(wv_sb[:], wvT[:])
            zmb_sb = big.tile([128, N], BF16)
            nc.sync.dma_start(zmb_sb[:], zmb[:])
            gbv_sb = big.tile([CO, 1], F32)
            nc.sync.dma_start(gbv_sb[:], gbv[:])
            ones_sb = big.tile([128, 128], BF16)
            nc.sync.dma_start(ones_sb[:], ones[:])

            # Two body copies per For_i iteration: the "dbl" pool rotates
            # between the copies, so consecutive iterations overlap (the
            # hardware loop replays a static instruction stream -- a single
            # body would serialize on its own head/tail tiles).
            from contextlib import nullcontext
            if repeat > 1 and repeat % 2 == 0:
                rep_ctx = tc.For_i(0, repeat // 2, 1)
                n_emit = 2
            elif repeat > 1:
                rep_ctx = tc.For_i(0, repeat, 1)
                n_emit = 1
            else:
                rep_ctx = nullcontext()
                n_emit = sim_unroll
            with rep_ctx:
                for _ in range(n_emit):
                    _emit_body(nc, tc, dbl, expp, work, pse, pspv,
                               zm_sb, zmb_sb, zq_sb, wqk_sb, bkq_sb,
                               wv_sb, gbv_sb, ones_sb, out)

    nc.finalize()
    return nc


def _emit_body(nc, tc, dbl, expp, work, pse, pspv,
               zm_sb, zmb_sb, zq_sb, wqk_sb, bkq_sb, wv_sb,
               gbv_sb, ones_sb, out):
    # residual (+ folded gamma*bv), exact fp32 bits of z_hsi
    zqp = dbl.tile([CH, NI], F32, tag="zqp")
    nc.vector.tensor_scalar_add(zqp[:], zq_sb[:].bitcast(F32), gbv_sb[:])

    # QK[c, i] = (Wk^T Wq zq + Wk^T bq)[c, i]   -> E = zm^T QK
    QK_sb = dbl.tile([128, NI], F32R, tag="qk")
    for c0 in range(0, NI, 400):
        cs = min(400, NI - c0)
        pq = pse.tile([128, 1536], F32, tag="e")
        nc.tensor.matmul(pq[:, :cs], wqk_sb[:], zq_sb[:, c0:c0 + cs],
                         start=True, stop=True)
        nc.vector.tensor_scalar_add(QK_sb[:, c0:c0 + cs], pq[:, :cs],
                                    bkq_sb[:])

    # VT tiles: VT[j, o] = sum_c zm[c, j] (gamma*Wv)[o, c] -- computed
    # interleaved into block 0's group loop (quad q lands at group q, always
    # ahead of the lag-3 PV consumer of tiles 3g..3g+2), so ACT's exp chain
    # starts immediately instead of idling behind a PE-only prefix.
    VT_sb = dbl.tile([128, JT * CO], BF16, tag="vt")

    # main attention loop; PV matmuls trail three exp-groups behind
    for i0, ibs in I_BLOCKS:
        pv = pspv.tile([128, 512], F32, tag="pv")
        sacc = [work.tile([128, 512], BF16, tag=f"s{c}", name=f"sacc{c}")
                for c in range(NCHAIN)]

        def emit_pv(p3_prev, gi):
            for t, jt in enumerate(GROUPS[gi]):
                nc.tensor.matmul(
                    pv[:, :ibs],
                    VT_sb[:, jt * 128:(jt + 1) * 128],
                    p3_prev[:, t * 512:t * 512 + ibs],
                    start=(jt == 0), stop=(jt == JT - 1),
                    skip_group_check=True)

        from collections import deque
        pending = deque()
        for gi, grp in enumerate(GROUPS):
            m = len(grp)
            e3 = pse.tile([128, 1536], F32, tag="e")
            for t, jt in enumerate(grp):
                nc.tensor.matmul(
                    e3[:, t * 512:t * 512 + ibs],
                    zm_sb[:, jt * 128:(jt + 1) * 128],
                    QK_sb[:, i0:i0 + ibs],
                    start=True, stop=True)
            p3 = expp.tile([128, 1536], BF16, tag="p")
            e3v = e3[:].rearrange("p (t x) -> p t x", x=512)[:, :m, :ibs]
            p3v = p3[:].rearrange("p (t x) -> p t x", x=512)[:, :m, :ibs]
            nc.scalar.activation(p3v, e3v, mybir.ActivationFunctionType.Exp)
            # denominator chains: NCHAIN independent bf16 accumulators
            for t in range(m):
                seg = p3[:, t * 512:t * 512 + ibs]
                if gi == 0:
                    nc.vector.tensor_copy(sacc[t][:, :ibs], seg)
                else:
                    nc.vector.tensor_add(sacc[t][:, :ibs],
                                         sacc[t][:, :ibs], seg)
            if i0 == 0 and gi * 4 < JT:
                nq = min(4, JT - gi * 4)
                vtq = pspv.tile([128, 512], F32, tag="d")
                for jj in range(nq):
                    j0 = (gi * 4 + jj) * 128
                    nc.tensor.matmul(vtq[:, jj * 128:(jj + 1) * 128],
                                     zmb_sb[:, j0:j0 + 128], wv_sb[:],
                                     start=True, stop=True,
                                     skip_group_check=True)
                nc.vector.tensor_copy(
                    VT_sb[:, gi * 512:gi * 512 + nq * 128],
                    vtq[:, :nq * 128])
            pending.append((p3, gi))
            if len(pending) > 3:
                emit_pv(*pending.popleft())
        while pending:
            emit_pv(*pending.popleft())

        # d[i] = sum_j exp: matmuls against the all-ones stationary reduce
        # the chains over partitions AND broadcast d[i] to all 128 output
        # partitions in one shot.
        dbc = pspv.tile([128, 512], F32, tag="d")
        for c in range(NCHAIN):
            nc.tensor.matmul(dbc[:, :ibs], ones_sb[:], sacc[c][:, :ibs],
                             start=(c == 0), stop=(c == NCHAIN - 1),
                             skip_group_check=True)

        # normalize: out = PV / d + zqp   (gamma pre-folded into Wv)
        r_sb = work.tile([128, 512], F32, tag="r")
        nc.vector.reciprocal(r_sb[:, :ibs], dbc[:, :ibs])
        t_sb = work.tile([128, 512], F32, tag="t")
        nc.vector.tensor_mul(t_sb[:, :ibs], pv[:, :ibs], r_sb[:, :ibs])
        o_sb = work.tile([128, 512], F32, tag="o")
        nc.vector.tensor_add(o_sb[:, :ibs], t_sb[:, :ibs],
                             zqp[:, i0:i0 + ibs])
        nc.sync.dma_start(out[:, i0:i0 + ibs], o_sb[:, :ibs])


_cached_nc = None


def kernel(z_hsi, z_msi, Wq, bq, Wk, bk, Wv, bv, gamma):
    global _cached_nc
    if _cached_nc is None:
        _cached_nc = _build()
    nc = _cached_nc

    z_hsi = np.asarray(z_hsi, dtype=np.float32).reshape(B, CH, N)
    z_msi = np.ascontiguousarray(np.asarray(z_msi, dtype=np.float32).reshape(B, CM, N))
    Wq64 = np.asarray(Wq, dtype=np.float64)
    Wk64 = np.asarray(Wk, dtype=np.float64)
    bq64 = np.asarray(bq, dtype=np.float64)
    # QK folding: E = zm^T (Wk^T Wq zq + Wk^T bq); bk cancels in softmax.
    # All CM=64 contractions are zero-padded to 128: K=64 matmuls run ~2x
    # slower per column on TRN2 than K=128.
    wqk_h = np.zeros((CH, 128), np.float32)
    wqk_h[:, :CM] = (Wq64.T @ Wk64).astype(np.float32)
    bkq_h = np.zeros((128, 1), np.float32)
    bkq_h[:CM, 0] = (Wk64.T @ bq64).astype(np.float32)
    g = float(np.asarray(gamma, dtype=np.float32).reshape(-1)[0])
    # gamma folds into Wv (bf16): out = (g*Wv) zm P / d + (z_hsi + g*bv)
    wvT_h = np.zeros((128, CO), ml_dtypes.bfloat16)
    wvT_h[:CM] = (g * np.asarray(Wv, np.float64).T).astype(np.float32)
    z_msi_pad = np.zeros((B, 128, N), np.float32)
    z_msi_pad[:, :CM] = z_msi
    z_msi_bf = z_msi_pad.astype(ml_dtypes.bfloat16)
    gbv = np.ascontiguousarray((g * np.asarray(bv, np.float32)).reshape(CO, 1))
    ones = np.ones((128, 128), dtype=ml_dtypes.bfloat16)

    shards_per_b = NCORES // B
    in_maps = []
    for c in range(NCORES):
        b, s = c // shards_per_b, (c % shards_per_b) * NI
        in_maps.append({
            "zq": np.ascontiguousarray(z_hsi[b][:, s:s + NI]),
            "zm": z_msi_pad[b], "zmb": z_msi_bf[b],
            "wqk": wqk_h, "bkq": bkq_h, "wvT": wvT_h,
            "gbv": gbv, "ones": ones,
        })

    res = run_bass_kernel_spmd(nc, in_maps, core_ids=list(range(NCORES)))

    out = np.empty((B, CH, N), dtype=np.float32)
    for c in range(NCORES):
        b, s = c // shards_per_b, (c % shards_per_b) * NI
        out[b][:, s:s + NI] = res.results[c]["out"]
    return out.reshape(B, CH, H, W)


# revision 24
# speedup vs baseline: 2.0792x; 1.4567x over previous
"""Cross-attention block (q from z_hsi, k/v from z_msi, softmax over 6400
pixels, residual + gamma) on 8 Trainium2 NeuronCores.

Sharding: the (batch=2, N=6400) query-pixel space is split into 8 shards of
1600 pixels (4 shards per batch element). Each core computes its shard's
attention output against the full key/value set of its batch element; the
host slices inputs and concatenates outputs (no device collectives).

Math restructuring vs the naive form:
  * softmax over j is invariant to adding any per-i constant, so the K bias
    (bk) is dropped entirely, and
      E[j,i] = K[:,j]^T Q[:,i]  ==  zm[:,j]^T (Wk^T Wq zq + Wk^T bq)[:,i]
    so K and Q are never materialized: one [64 x 1600] "QK" projection
    (host precomputes Wq^T Wk and Wk^T bq) replaces both.
  * The V bias folds out of the attention matmul:  (V+bv) P = V P + bv * d,
    so after normalizing by d it becomes "+ gamma*bv" on the residual;
    gamma itself is folded into Wv on the host, so the device only ever
    divides by the raw denominator d.
  * E/QK matmuls run in float32r (TF32-like precision at full PE rate,
    1 cycle/row for moving widths >= 256).  exp() output is bfloat16:
    ACT throughput is dtype-independent, but bf16 gives the PV matmul a
    1 cycle/row moving operand at ANY width and doubles DVE throughput
    (2x_1p mode) for the denominator accumulation.
  * All K=64 contractions (z_msi channels) are zero-padded to K=128.
  * Denominator: instead of one ones-matmul per exp pair (~17us of PE),
    two bf16 DVE chain-accumulators per i-block sum the 50 exp tiles
    elementwise; a single pair of matmuls against an all-ones [128,128]
    stationary then reduces over partitions AND broadcasts d[i] to all
    128 output partitions in one shot (so no GPSIMD broadcast either).
  * exp runs on ACT straight out of PSUM in [128, 2x400] strided batches;
    PV matmuls trail three exp-groups behind (software pipeline).  VT
    tiles (V^T = zm^T Wv) are computed on the PE interleaved into block
    0's group loop; their PSUM->SBUF(bf16) copies run on the otherwise
    idle GPSIMD/Pool engine.
"""
import sys

sys.path.insert(0, "/opt/trn_rl_repo")

import ml_dtypes
import numpy as np
import concourse.bass as bass  # noqa: F401
import concourse.tile as tile
from concourse import bacc, mybir
from concourse.bass_utils import run_bass_kernel_spmd

B, CH, CM, CO = 2, 128, 64, 128
H = W = 80
N = H * W                # 6400 key/value pixels per batch element
NCORES = 8
NI = (B * N) // NCORES   # 1600 query pixels per core
JT = N // 128            # 50 key tiles
F32 = mybir.dt.float32
F32R = mybir.dt.float32r
BF16 = mybir.dt.bfloat16

IBS = 400
I_BLOCKS = [(k * IBS, IBS) for k in range(NI // IBS)]
# jt groups of 3: one exp instruction per group
GROUPS = [tuple(range(g, min(g + 3, JT))) for g in range(0, JT, 3)]
NCHAIN = 3


def _build(repeat=1, sim_unroll=1):
    """repeat>1 wraps the per-core compute in an on-device For_i loop (two
    body copies per iteration); used only by the perf harness to measure HW
    time via wall-clock slope.  sim_unroll emits extra python-unrolled body
    copies at repeat=1 for TimelineSim marginal-cost analysis."""
    nc = bacc.Bacc(None, target_bir_lowering=False)
    zq = nc.declare_dram_parameter("zq", [CH, NI], F32R, isOutput=False)
    zm = nc.declare_dram_parameter("zm", [128, N], F32R, isOutput=False)
    wqk = nc.declare_dram_parameter("wqk", [CH, 128], F32R, isOutput=False)
    bkq = nc.declare_dram_parameter("bkq", [128, 1], F32, isOutput=False)
    wvT = nc.declare_dram_parameter("wvT", [128, CO], BF16, isOutput=False)
    zmb = nc.declare_dram_parameter("zmb", [128, N], BF16, isOutput=False)
    gbv = nc.declare_dram_parameter("gbv", [CO, 1], F32, isOutput=False)
    ones = nc.declare_dram_parameter("ones", [128, 128], BF16, isOutput=False)
    out = nc.declare_dram_parameter("out", [CO, NI], F32, isOutput=True)

    with tile.TileContext(nc) as tc:
        with (
            tc.tile_pool(name="big", bufs=1) as big,
            tc.tile_pool(name="dbl", bufs=2) as dbl,
            tc.tile_pool(name="expp", bufs=6) as expp,
            tc.tile_pool(name="work", bufs=2) as work,
            tc.tile_pool(name="pse", bufs=2, space="PSUM") as pse,
            tc.tile_pool(name="pspv", bufs=1, space="PSUM") as pspv,
        ):
            zm_sb = big.tile([128, N], F32R)
            nc.sync.dma_start(zm_sb[:], zm[:])
            zq_sb = big.tile([CH, NI], F32R)
            nc.sync.dma_start(zq_sb[:], zq[:])
            wqk_sb = big.tile([CH, 128], F32R)
            nc.sync.dma_start(wqk_sb[:], wqk[:])
            bkq_sb = big.tile([128, 1], F32)
            nc.sync.dma_start(bkq_sb[:], bkq[:])
            wv_sb = big.tile([128, CO], BF16)
            nc.sync.dma_start(wv_sb[:], wvT[:])
            zmb_sb = big.tile([128, N], BF16)
            nc.sync.dma_start(zmb_sb[:], zmb[:])
            gbv_sb = big.tile([CO, 1], F32)
            nc.sync.dma_start(gbv_sb[:], gbv[:])
            ones_sb = big.tile([128, 128], BF16)
            nc.sync.dma_start(ones_sb[:], ones[:])

            # Two body copies per For_i iteration: the "dbl" pool rotates
            # between the copies, so consecutive iterations overlap (the
            # hardware loop replays a static instruction stream -- a single
            # body would serialize on its own head/tail tiles).
            from contextlib import nullcontext
            if repeat > 1 and repeat % 2 == 0:
                rep_ctx = tc.For_i(0, repeat // 2, 1)
                n_emit = 2
            elif repeat > 1:
                rep_ctx = tc.For_i(0, repeat, 1)
                n_emit = 1
            else:
                rep_ctx = nullcontext()
                n_emit = sim_unroll
            with rep_ctx:
                for _ in range(n_emit):
                    _emit_body(nc, tc, dbl, expp, work, pse, pspv,
                               zm_sb, zmb_sb, zq_sb, wqk_sb, bkq_sb,
                               wv_sb, gbv_sb, ones_sb, out)

    nc.finalize()
    return nc


def _emit_body(nc, tc, dbl, expp, work, pse, pspv,
               zm_sb, zmb_sb, zq_sb, wqk_sb, bkq_sb, wv_sb,
               gbv_sb, ones_sb, out):
    # residual (+ folded gamma*bv), exact fp32 bits of z_hsi
    zqp = dbl.tile([CH, NI], F32, tag="zqp")
    nc.vector.tensor_scalar_add(zqp[:], zq_sb[:].bitcast(F32), gbv_sb[:])

    # QK[c, i] = (Wk^T Wq zq + Wk^T bq)[c, i]   -> E = zm^T QK
    QK_sb = dbl.tile([128, NI], F32R, tag="qk")
    for c0 in range(0, NI, 400):
        cs = min(400, NI - c0)
        pq = pse.tile([128, 1536], F32, tag="e")
        nc.tensor.matmul(pq[:, :cs], wqk_sb[:], zq_sb[:, c0:c0 + cs],
                         start=True, stop=True)
        nc.vector.tensor_scalar_add(QK_sb[:, c0:c0 + cs], pq[:, :cs],
                                    bkq_sb[:])

    # VT tiles: VT[j, o] = sum_c zm[c, j] (gamma*Wv)[o, c] -- computed
    # interleaved into block 0's group loop (quad q lands at group q, always
    # ahead of the lag-3 PV consumer of tiles 3g..3g+2), so ACT's exp chain
    # starts immediately instead of idling behind a PE-only prefix.
    VT_sb = dbl.tile([128, JT * CO], BF16, tag="vt")

    # Main attention loop: ONE flat software pipeline over all
    # (block, group) pairs.  PV matmuls trail the exp stream by two group
    # slots; a block's last PV (and its denominator/normalize tail) thus
    # lands two slots into the NEXT block, after that block's E-matmuls --
    # ACT's exp stream never waits behind block-boundary bookkeeping.
    # Right after a block's last PV, its PSUM accumulator is copied to
    # SBUF, releasing the single pv bank ~3 DVE-ops earlier than the
    # normalize chain would; the normalize then runs entirely SBUF-side.
    from collections import deque
    pending = deque()
    blk = {}

    def emit_pv(bi, gi, p3):
        pv, sacc = blk[bi]
        i0, ibs = I_BLOCKS[bi]
        for t, jt in enumerate(GROUPS[gi]):
            nc.tensor.matmul(
                pv[:, :ibs],
                VT_sb[:, jt * 128:(jt + 1) * 128],
                p3[:, t * 512:t * 512 + ibs],
                start=(jt == 0), stop=(jt == JT - 1),
                skip_group_check=True)
        if gi == len(GROUPS) - 1:
            emit_tail(bi)

    def emit_tail(bi):
        pv, sacc = blk.pop(bi)
        i0, ibs = I_BLOCKS[bi]
        # free the pv PSUM bank immediately: raw copy to SBUF
        pvs = work.tile([128, 512], F32, tag="pvs")
        nc.vector.tensor_copy(pvs[:, :ibs], pv[:, :ibs])
        # d[i] = sum_j exp: matmuls against the all-ones stationary reduce
        # the chains over partitions AND broadcast d[i] to all 128 output
        # partitions in one shot.
        dbc = pspv.tile([128, 512], F32, tag="d")
        for c in range(NCHAIN):
            nc.tensor.matmul(dbc[:, :ibs], ones_sb[:], sacc[c][:, :ibs],
                             start=(c == 0), stop=(c == NCHAIN - 1),
                             skip_group_check=True)
        # normalize: out = PV / d + zqp  (gamma pre-folded into Wv)
        r_sb = work.tile([128, 512], F32, tag="r")
        nc.vector.reciprocal(r_sb[:, :ibs], dbc[:, :ibs])
        t_sb = work.tile([128, 512], F32, tag="t")
        nc.vector.tensor_mul(t_sb[:, :ibs], pvs[:, :ibs], r_sb[:, :ibs])
        o_sb = work.tile([128, 512], F32, tag="o")
        nc.vector.tensor_add(o_sb[:, :ibs], t_sb[:, :ibs],
                             zqp[:, i0:i0 + ibs])
        nc.sync.dma_start(out[:, i0:i0 + ibs], o_sb[:, :ibs])

    for bi, (i0, ibs) in enumerate(I_BLOCKS):
        for gi, grp in enumerate(GROUPS):
            if gi == 0:
                pv = pspv.tile([128, 512], F32, tag="pv")
                sacc = work.tile([128, NCHAIN * 512], BF16, tag="sacc")
                blk[bi] = (pv, sacc)
            pv, sacc = blk[bi]
            m = len(grp)
            e3 = pse.tile([128, 1536], F32, tag="e")
            for t, jt in enumerate(grp):
                nc.tensor.matmul(
                    e3[:, t * 512:t * 512 + ibs],
                    zm_sb[:, jt * 128:(jt + 1) * 128],
                    QK_sb[:, i0:i0 + ibs],
                    start=True, stop=True)
            p3 = expp.tile([128, 1536], BF16, tag="p")
            e3v = e3[:].rearrange("p (t x) -> p t x", x=512)[:, :m, :ibs]
            p3v = p3[:].rearrange("p (t x) -> p t x", x=512)[:, :m, :ibs]
            nc.scalar.activation(p3v, e3v, mybir.ActivationFunctionType.Exp)
            # denominator chains: NCHAIN independent bf16 accumulators
            for t in range(m):
                seg = p3[:, t * 512:t * 512 + ibs]
                if gi == 0:
                    nc.vector.tensor_copy(sacc[t][:, :ibs], seg)
                else:
                    nc.vector.tensor_add(sacc[t][:, :ibs],
                                         sacc[t][:, :ibs], seg)
            if i0 == 0 and gi * 4 < JT:
                nq = min(4, JT - gi * 4)
                vtq = pspv.tile([128, 512], F32, tag="d")
                for jj in range(nq):
                    j0 = (gi * 4 + jj) * 128
                    nc.tensor.matmul(vtq[:, jj * 128:(jj + 1) * 128],
                                     zmb_sb[:, j0:j0 + 128], wv_sb[:],
                                     start=True, stop=True,
                                     skip_group_check=True)
                nc.vector.tensor_copy(
                    VT_sb[:, gi * 512:gi * 512 + nq * 128],
                    vtq[:, :nq * 128])
            pending.append((bi, gi, p3))
            if len(pending) > 3:
                emit_pv(*pending.popleft())
    while pending:
        emit_pv(*pending.popleft())


_cached_nc = None


def kernel(z_hsi, z_msi, Wq, bq, Wk, bk, Wv, bv, gamma):
    global _cached_nc
    if _cached_nc is None:
        _cached_nc = _build()
    nc = _cached_nc

    z_hsi = np.asarray(z_hsi, dtype=np.float32).reshape(B, CH, N)
    z_msi = np.ascontiguousarray(np.asarray(z_msi, dtype=np.float32).reshape(B, CM, N))
    Wq64 = np.asarray(Wq, dtype=np.float64)
    Wk64 = np.asarray(Wk, dtype=np.float64)
    bq64 = np.asarray(bq, dtype=np.float64)
    # QK folding: E = zm^T (Wk^T Wq zq + Wk^T bq); bk cancels in softmax.
    # All CM=64 contractions are zero-padded to 128: K=64 matmuls run ~2x
    # slower per column on TRN2 than K=128.
    wqk_h = np.zeros((CH, 128), np.float32)
    wqk_h[:, :CM] = (Wq64.T @ Wk64).astype(np.float32)
    bkq_h = np.zeros((128, 1), np.float32)
    bkq_h[:CM, 0] = (Wk64.T @ bq64).astype(np.float32)
    g = float(np.asarray(gamma, dtype=np.float32).reshape(-1)[0])
    # gamma folds into Wv (bf16): out = (g*Wv) zm P / d + (z_hsi + g*bv)
    wvT_h = np.zeros((128, CO), ml_dtypes.bfloat16)
    wvT_h[:CM] = (g * np.asarray(Wv, np.float64).T).astype(np.float32)
    z_msi_pad = np.zeros((B, 128, N), np.float32)
    z_msi_pad[:, :CM] = z_msi
    z_msi_bf = z_msi_pad.astype(ml_dtypes.bfloat16)
    gbv = np.ascontiguousarray((g * np.asarray(bv, np.float32)).reshape(CO, 1))
    ones = np.ones((128, 128), dtype=ml_dtypes.bfloat16)

    shards_per_b = NCORES // B
    in_maps = []
    for c in range(NCORES):
        b, s = c // shards_per_b, (c % shards_per_b) * NI
        in_maps.append({
            "zq": np.ascontiguousarray(z_hsi[b][:, s:s + NI]),
            "zm": z_msi_pad[b], "zmb": z_msi_bf[b],
            "wqk": wqk_h, "bkq": bkq_h, "wvT": wvT_h,
            "gbv": gbv, "ones": ones,
        })

    res = run_bass_kernel_spmd(nc, in_maps, core_ids=list(range(NCORES)))

    out = np.empty((B, CH, N), dtype=np.float32)
    for c in range(NCORES):
        b, s = c // shards_per_b, (c % shards_per_b) * NI
        out[b][:, s:s + NI] = res.results[c]["out"]
    return out.reshape(B, CH, H, W)
